# revision 16
# baseline (speedup 1.0000x reference)
"""Trainium2 Bass kernel for nn_DeepCluster (vq_codebook).

Math (per row x in R^72):
  7-layer MLP, ReLU only after layers 2 and 4  ->  f in R^200
  sq[j] = |f - center[:, j]|^2 ;  q = (1/(1+sq)) / sum_j (1/(1+sq))

Because ReLU appears only after layers 2 and 4, the layer chains 1-2,
3-4 and 5-6-7 are affine compositions.  The host pre-multiplies them in
float64 into three matrices W12 [72,256], W34 [256,512], W567 [512,200]
(3.25x fewer matmul FLOPs than the literal 7-layer chain).  The device
then computes, per 512-row tile (feature-major layout [feat, batch]):

  A: h2 = relu(x @ W12 + b12)        2 bf16 matmuls (bias via ones-row)
  B: h4 = relu(h2 @ W34 + b34)       4 fp8 DoubleRow matmuls (K=256)
  C: e  = h4 @ W567                  4 fp8 DoubleRow matmuls (K=512)
  D: sq = |e|^2 - 2 e.(c-b567) ...   2 fp8 DoubleRow matmuls (K=200)
  tail: 1/(1+sq), transpose back, row-normalize, store

The C-layer bias b567 is folded into shifted centers c' = c - b567 and
|c'|^2, so B is the only stage needing per-feature bias epilogues.
Activations are scaled by per-stage powers of two (calibrated on a host
subsample) to sit in fp8e4 range; all scale folds are exact.
Work is software-pipelined 3 tiles deep and spread over ACT/DVE/GPSIMD
so the PE (12 matmuls + 4 transposes per tile) stays the critical path.
"""

import numpy as np

N_CORES = 8
B = 512  # rows per pipeline tile
P = 128

_CACHE = {}


def _build(n_rows, cA, sC, sD):
    import concourse.bass as bass
    import concourse.mybir as mybir
    from concourse import bacc
    from concourse.tile import TileContext

    f32 = mybir.dt.float32
    bf16 = mybir.dt.bfloat16
    fp8 = mybir.dt.float8e4
    AF = mybir.ActivationFunctionType
    AX = mybir.AxisListType
    ALU = mybir.AluOpType
    DR = mybir.MatmulPerfMode.DoubleRow

    nc = bacc.Bacc(None, target_bir_lowering=False, debug=False)
    xt_d = nc.dram_tensor("xt", [73, n_rows], bf16, kind="ExternalInput")
    q_d = nc.dram_tensor("q", [n_rows, 72], f32, kind="ExternalOutput")
    w12_d = nc.dram_tensor("w12", [73, 256], bf16, kind="ExternalInput")
    w34_d = nc.dram_tensor("w34", [128, 1024], fp8, kind="ExternalInput")
    w567_d = nc.dram_tensor("w567", [128, 896], fp8, kind="ExternalInput")
    cm2_d = nc.dram_tensor("cm2", [100, 160], fp8, kind="ExternalInput")
    onesd_d = nc.dram_tensor("onesd", [100, 160], fp8, kind="ExternalInput")
    csq_d = nc.dram_tensor("csq", [72, 1], f32, kind="ExternalInput")

    n_tiles = n_rows // B
    assert n_rows % B == 0
    C = B // P

    with TileContext(nc) as tc:
        with (
            tc.tile_pool(name="consts", bufs=1) as consts,
            tc.tile_pool(name="xt", bufs=4) as xtp,
            tc.tile_pool(name="acts", bufs=2) as acts,
            tc.tile_pool(name="h2p", bufs=3) as h2p,
            tc.tile_pool(name="fg", bufs=5) as fgp,
            tc.tile_pool(name="pmm", bufs=3, space="PSUM") as pmm,
            tc.tile_pool(name="pd", bufs=2, space="PSUM") as pdp,
        ):
            w12 = consts.tile([73, 256], bf16, tag="w12")
            nc.sync.dma_start(out=w12, in_=w12_d[:])
            w34 = consts.tile([128, 2, 512], fp8, tag="w34")
            nc.sync.dma_start(out=w34, in_=w34_d[:].rearrange("p (i m) -> p i m", i=2))
            w567 = consts.tile([128, 4, 224], fp8, tag="w567")
            nc.sync.dma_start(
                out=w567, in_=w567_d[:].rearrange("p (i m) -> p i m", i=4)
            )
            cm2 = consts.tile([100, 2, 80], fp8, tag="cm2")
            nc.sync.dma_start(out=cm2, in_=cm2_d[:].rearrange("p (i m) -> p i m", i=2))
            onesd = consts.tile([100, 2, 80], fp8, tag="onesd")
            nc.sync.dma_start(
                out=onesd, in_=onesd_d[:].rearrange("p (i m) -> p i m", i=2)
            )
            csq = consts.tile([72, 1], f32, tag="csq")
            nc.sync.dma_start(out=csq, in_=csq_d[:])

            q_r = q_d[:].rearrange("(t s p) j -> t p s j", p=P, s=C)

            xt_sb = [None] * n_tiles
            h2_sb = [None] * n_tiles
            h4_sb = [None] * n_tiles
            f_sb = [None] * n_tiles
            g_sb = [None] * n_tiles
            sd_sb = [None] * n_tiles
            nom_sb = [None] * n_tiles

            def load(t):
                xt_sb[t] = xtp.tile([73, B], bf16, name="xt", tag="x")
                nc.sync.dma_start(out=xt_sb[t], in_=xt_d[:, B * t : B * (t + 1)])

            def stageA(t):
                ps = pmm.tile([128, 2, B], f32, name="psmm", tag="mm")
                for m in range(2):
                    nc.tensor.matmul(
                        ps[:, m, :], w12[:, 128 * m : 128 * (m + 1)], xt_sb[t],
                        start=True, stop=True,
                    )
                h2_sb[t] = h2p.tile([128, 2, B], fp8, name="h2", tag="h2")
                nc.scalar.activation(
                    out=h2_sb[t], in_=ps, func=AF.Relu, bias=0.0, scale=cA
                )
                xt_sb[t] = None

            def stageB(t):
                pss = []
                for half in range(2):
                    ps = pmm.tile([128, 2, B], f32, name="psmm", tag="mm")
                    for mi in range(2):
                        m = 2 * half + mi
                        nc.tensor.matmul(
                            ps[:, mi, :],
                            w34[:, :, 128 * m : 128 * (m + 1)],
                            h2_sb[t],
                            start=True, stop=True, perf_mode=DR,
                        )
                    pss.append(ps)
                h4_sb[t] = acts.tile([128, 4, B], fp8, name="h4", tag="h4", bufs=3)
                nc.scalar.activation(
                    out=h4_sb[t][:, 0:2, :], in_=pss[0], func=AF.Relu,
                    bias=0.0, scale=1.0,
                )
                nc.vector.tensor_scalar_max(h4_sb[t][:, 2:4, :], pss[1], 0.0)
                h2_sb[t] = None

            def stageC(t):
                ps = pmm.tile([100, 2, B], f32, name="psc", tag="mm")
                for c in range(2):
                    for mp in range(2):
                        nc.tensor.matmul(
                            ps[:, mp, :],
                            w567[:, 2 * c : 2 * c + 2, 112 * mp : 112 * mp + 100],
                            h4_sb[t][:, 2 * c : 2 * c + 2, :],
                            start=(c == 0), stop=(c == 1), perf_mode=DR,
                        )
                f_sb[t] = fgp.tile([100, 2, B], fp8, name="ft", tag="f")
                nc.scalar.activation(
                    out=f_sb[t], in_=ps, func=AF.Identity, bias=0.0, scale=sC
                )
                g_sb[t] = fgp.tile([100, 2, B], fp8, name="gt", tag="g")
                nc.gpsimd.tensor_mul(g_sb[t], f_sb[t], f_sb[t])
                h4_sb[t] = None

            def stageD(t):
                ps = pdp.tile([72, B], f32, name="psd", tag="sd")
                nc.tensor.matmul(
                    ps, cm2[:, :, 0:72], f_sb[t], start=True, stop=False, perf_mode=DR
                )
                nc.tensor.matmul(
                    ps, onesd[:, :, 0:72], g_sb[t], start=False, stop=True, perf_mode=DR
                )
                sd = acts.tile([80, B], bf16, name="sd", tag="sdp")
                nc.vector.tensor_scalar(
                    out=sd[0:72, :], in0=ps, scalar1=sD, scalar2=csq[:, 0:1],
                    op0=ALU.mult, op1=ALU.add,
                )
                nomT = acts.tile([80, B], bf16, name="nomT", tag="nomT")
                with nc.allow_low_precision("q tolerance 2e-2; bf16 nom is ~4e-3"):
                    nc.vector.reciprocal(out=nomT[0:72, :], in_=sd[0:72, :])
                nom_sb[t] = acts.tile([P, C, 80], bf16, name="nom", tag="nom")
                nc.sync.dma_start_transpose(nom_sb[t], nomT)
                f_sb[t] = None
                g_sb[t] = None

            def tail(t):
                nom = nom_sb[t][:, :, 0:72]
                rs = acts.tile([P, C], f32, name="rs", tag="rs")
                nc.vector.reduce_sum(rs, nom, axis=AX.X)
                rr = acts.tile([P, C], f32, name="rr", tag="rr")
                nc.vector.reciprocal(rr, rs)
                rr_b = bass.AP(
                    tensor=rr.tensor,
                    offset=rr.offset,
                    ap=[rr.ap[0], rr.ap[1], [0, 72]],
                )
                qt = acts.tile([P, C, 72], f32, name="qt", tag="qt")
                nc.gpsimd.tensor_tensor(out=qt, in0=nom, in1=rr_b, op=ALU.mult)
                nc.sync.dma_start(out=q_r[t], in_=qt)
                nom_sb[t] = None

            load(0)
            load(1)
            stageA(0)
            load(2)
            stageA(1)
            for t in range(n_tiles + 4):
                if t + 3 < n_tiles:
                    load(t + 3)
                if 0 <= t - 2 < n_tiles:
                    stageC(t - 2)
                if 0 <= t - 4:
                    tail(t - 4)
                if t < n_tiles:
                    stageB(t)
                if 0 <= t - 3 < n_tiles:
                    stageD(t - 3)
                if t + 2 < n_tiles:
                    stageA(t + 2)

    nc.compile()
    return nc


def _pow2(v):
    return float(2.0 ** np.round(np.log2(v)))


def prepare(inputs_np):
    """Host-side marshalling: merge affine chains in f64, calibrate fp8
    scales on a subsample, quantize, build per-core input maps."""
    import ml_dtypes

    bf = ml_dtypes.bfloat16
    f8 = ml_dtypes.float8_e4m3

    x = np.asarray(inputs_np["inputs"], dtype=np.float64)
    ws = [np.asarray(inputs_np[f"w{i}"], dtype=np.float64) for i in range(1, 8)]
    bs = [np.asarray(inputs_np[f"b{i}"], dtype=np.float64) for i in range(1, 8)]
    center = np.asarray(inputs_np["center"], dtype=np.float64)

    W12 = ws[0] @ ws[1]
    b12 = bs[0] @ ws[1] + bs[1]
    W34 = ws[2] @ ws[3]
    b34 = bs[2] @ ws[3] + bs[3]
    W567 = ws[4] @ ws[5] @ ws[6]
    b567 = (bs[4] @ ws[5] + bs[5]) @ ws[6] + bs[6]
    cp = center - b567[:, None]  # shifted centers c' = c - b567, [200, 72]

    n = x.shape[0]
    sub = x[:: max(1, n // 4096)][:4096]
    h2 = np.maximum(sub @ W12 + b12, 0.0)
    h4 = np.maximum(h2 @ W34 + b34, 0.0)
    e = h4 @ W567

    def rms(a):
        return float(np.sqrt(np.mean(a.astype(np.float64) ** 2)) + 1e-30)

    cA = _pow2(1.0 / rms(h2))
    kB = _pow2(0.25 / rms(W34))
    # keep the (kB*cA)-scaled h4 inside fp8 range
    while kB * cA * rms(h4) > 8.0:
        kB /= 2.0
    kC = _pow2(0.25 / rms(W567))
    cF = min(_pow2(1.0 / rms(e)), 256.0)
    sC = cF / (kC * kB * cA)
    sD = 1.0 / cF  # kD == cF

    def q8(a):
        return np.clip(a, -224.0, 224.0).astype(f8)

    # Drop the single least-impactful h2 feature to free one slot in the
    # 256-wide h2 vector; the freed slot carries a constant so b34 rides
    # through the B matmul and its epilogues need no bias columns.
    impact = np.mean(h2**2, axis=0) * np.sum(W34**2, axis=1)
    jstar = int(np.argmin(impact))
    keep = [j for j in range(256) if j != jstar]
    W12k = W12[:, keep]
    b12k = b12[keep]
    W34k = W34[keep, :]

    consts = {}
    w12t = np.zeros((73, 256), dtype=np.float64)
    w12t[:72, 0:255] = W12k
    w12t[72, 0:255] = b12k
    w12t[72, 255] = 1.0  # ones-slot: psA[255] = 1 -> h2'[255] = cA exactly
    consts["w12"] = w12t.astype(bf)
    w34f = np.zeros((256, 512), dtype=np.float64)
    w34f[0:255, :] = kB * W34k
    w34f[255, :] = kB * b34  # h2'[255] = cA  =>  contributes kB*cA*b34
    w34t = np.zeros((128, 2, 512), dtype=np.float64)
    for i in range(2):
        w34t[:, i, :] = w34f[128 * i : 128 * (i + 1), :]
    consts["w34"] = q8(w34t.reshape(128, 1024))
    w567t = np.zeros((128, 4, 224), dtype=np.float64)
    for c in range(2):
        for i in range(2):
            blk = kC * W567[256 * c + 128 * i : 256 * c + 128 * (i + 1), :]
            w567t[:, 2 * c + i, 0:100] = blk[:, 0:100]
            w567t[:, 2 * c + i, 112:212] = blk[:, 100:200]
    consts["w567"] = q8(w567t.reshape(128, 896))
    cm2t = np.zeros((100, 2, 80), dtype=np.float64)
    onest = np.zeros((100, 2, 80), dtype=np.float64)
    for i in range(2):
        cm2t[:, i, 0:72] = -2.0 * cp[100 * i : 100 * (i + 1), :]
        onest[:, i, 0:72] = 1.0 / cF
    consts["cm2"] = q8(cm2t.reshape(100, 160))
    consts["onesd"] = q8(onest.reshape(100, 160))
    consts["csq"] = (1.0 + (cp**2).sum(axis=0)).reshape(72, 1).astype(np.float32)

    n_loc = n // N_CORES
    key = (n_loc, cA, sC, sD)
    if key not in _CACHE:
        _CACHE[key] = _build(n_loc, cA, sC, sD)
    nc = _CACHE[key]

    in_maps = []
    for c in range(N_CORES):
        xt = np.empty((73, n_loc), dtype=bf)
        xt[:72] = x[c * n_loc : (c + 1) * n_loc].T
        xt[72] = 1.0
        m = {"xt": np.ascontiguousarray(xt)}
        m.update(consts)
        in_maps.append(m)
    return nc, in_maps


def kernel(
    inputs, w1, b1, w2, b2, w3, b3, w4, b4, w5, b5, w6, b6, w7, b7, center
):
    from concourse.bass_utils import run_bass_kernel_spmd

    inputs_np = {
        "inputs": inputs, "center": center,
        "w1": w1, "b1": b1, "w2": w2, "b2": b2, "w3": w3, "b3": b3,
        "w4": w4, "b4": b4, "w5": w5, "b5": b5, "w6": w6, "b6": b6,
        "w7": w7, "b7": b7,
    }
    nc, in_maps = prepare(inputs_np)
    res = run_bass_kernel_spmd(nc, in_maps, core_ids=list(range(N_CORES)))
    return np.concatenate([res.results[c]["q"] for c in range(N_CORES)], axis=0)


# revision 17
# speedup vs baseline: 1.9183x; 1.9183x over previous
"""Trainium2 Bass kernel for nn_DeepCluster (vq_codebook).

Math (per row x in R^72):
  7-layer MLP, ReLU only after layers 2 and 4  ->  f in R^200
  sq[j] = |f - center[:, j]|^2 ;  q = (1/(1+sq)) / sum_j (1/(1+sq))

Because ReLU appears only after layers 2 and 4, the layer chains 1-2,
3-4 and 5-6-7 are affine compositions.  The host pre-multiplies them in
float64 into three matrices W12 [72,256], W34 [256,512], W567 [512,200]
(3.25x fewer matmul FLOPs than the literal 7-layer chain).  The device
then computes, per 512-row tile (feature-major layout [feat, batch]):

  A: h2 = relu(x @ W12 + b12)        2 bf16 matmuls (bias via ones-row)
  B: h4 = relu(h2 @ W34 + b34)       4 fp8 DoubleRow matmuls (K=256)
  C: e  = h4 @ W567                  4 fp8 DoubleRow matmuls (K=512)
  D: sq = |e|^2 - 2 e.(c-b567) ...   2 fp8 DoubleRow matmuls (K=200)
  tail: 1/(1+sq), transpose back, row-normalize, store

The C-layer bias b567 is folded into shifted centers c' = c - b567 and
|c'|^2, so B is the only stage needing per-feature bias epilogues.
Activations are scaled by per-stage powers of two (calibrated on a host
subsample) to sit in fp8e4 range; all scale folds are exact.
Work is software-pipelined 3 tiles deep and spread over ACT/DVE/GPSIMD
so the PE (12 matmuls + 4 transposes per tile) stays the critical path.
"""

import numpy as np

N_CORES = 8
B = 512  # rows per pipeline tile
P = 128

_CACHE = {}


def _build(n_rows, cA, sC, sD):
    import concourse.bass as bass
    import concourse.mybir as mybir
    from concourse import bacc
    from concourse.tile import TileContext
    from concourse.masks import make_identity

    f32 = mybir.dt.float32
    bf16 = mybir.dt.bfloat16
    fp8 = mybir.dt.float8e4
    AF = mybir.ActivationFunctionType
    AX = mybir.AxisListType
    ALU = mybir.AluOpType
    DR = mybir.MatmulPerfMode.DoubleRow

    nc = bacc.Bacc(None, target_bir_lowering=False, debug=False)
    xt_d = nc.dram_tensor("xt", [73, n_rows], bf16, kind="ExternalInput")
    q_d = nc.dram_tensor("q", [n_rows, 72], f32, kind="ExternalOutput")
    w12_d = nc.dram_tensor("w12", [73, 256], bf16, kind="ExternalInput")
    w34_d = nc.dram_tensor("w34", [128, 1024], fp8, kind="ExternalInput")
    w567_d = nc.dram_tensor("w567", [128, 896], fp8, kind="ExternalInput")
    cm2_d = nc.dram_tensor("cm2", [100, 160], fp8, kind="ExternalInput")
    onesd_d = nc.dram_tensor("onesd", [100, 160], fp8, kind="ExternalInput")
    csq_d = nc.dram_tensor("csq", [72, 1], f32, kind="ExternalInput")

    n_tiles = n_rows // B
    assert n_rows % B == 0
    C = B // P

    with TileContext(nc) as tc:
        with (
            tc.tile_pool(name="consts", bufs=1) as consts,
            tc.tile_pool(name="xt", bufs=4) as xtp,
            tc.tile_pool(name="acts", bufs=2) as acts,
            tc.tile_pool(name="h2p", bufs=3) as h2p,
            tc.tile_pool(name="fg", bufs=5) as fgp,
            tc.tile_pool(name="pmm", bufs=3, space="PSUM") as pmm,
            tc.tile_pool(name="pd", bufs=1, space="PSUM") as pdp,
            tc.tile_pool(name="pt", bufs=1, space="PSUM") as ptp,
        ):
            identf = consts.tile([128, 128], f32, tag="identf")
            make_identity(nc, identf)
            w12 = consts.tile([73, 256], bf16, tag="w12")
            nc.sync.dma_start(out=w12, in_=w12_d[:])
            w34 = consts.tile([128, 2, 512], fp8, tag="w34")
            nc.sync.dma_start(out=w34, in_=w34_d[:].rearrange("p (i m) -> p i m", i=2))
            w567 = consts.tile([128, 4, 224], fp8, tag="w567")
            nc.sync.dma_start(
                out=w567, in_=w567_d[:].rearrange("p (i m) -> p i m", i=4)
            )
            cm2 = consts.tile([100, 2, 80], fp8, tag="cm2")
            nc.sync.dma_start(out=cm2, in_=cm2_d[:].rearrange("p (i m) -> p i m", i=2))
            onesd = consts.tile([100, 2, 80], fp8, tag="onesd")
            nc.sync.dma_start(
                out=onesd, in_=onesd_d[:].rearrange("p (i m) -> p i m", i=2)
            )
            csq = consts.tile([72, 1], f32, tag="csq")
            nc.sync.dma_start(out=csq, in_=csq_d[:])

            q_r = q_d[:].rearrange("(t s p) j -> t p s j", p=P, s=C)

            xt_sb = [None] * n_tiles
            h2_sb = [None] * n_tiles
            h4_sb = [None] * n_tiles
            f_sb = [None] * n_tiles
            g_sb = [None] * n_tiles
            sd_sb = [None] * n_tiles
            nom_sb = [None] * n_tiles

            def load(t):
                xt_sb[t] = xtp.tile([73, B], bf16, name="xt", tag="x")
                nc.sync.dma_start(out=xt_sb[t], in_=xt_d[:, B * t : B * (t + 1)])

            def stageA(t):
                ps = pmm.tile([128, 2, B], f32, name="psmm", tag="mm")
                for m in range(2):
                    nc.tensor.matmul(
                        ps[:, m, :], w12[:, 128 * m : 128 * (m + 1)], xt_sb[t],
                        start=True, stop=True,
                    )
                h2_sb[t] = h2p.tile([128, 2, B], fp8, name="h2", tag="h2")
                nc.scalar.activation(
                    out=h2_sb[t], in_=ps, func=AF.Relu, bias=0.0, scale=cA
                )
                xt_sb[t] = None

            def stageB(t):
                pss = []
                for half in range(2):
                    ps = pmm.tile([128, 2, B], f32, name="psmm", tag="mm")
                    for mi in range(2):
                        m = 2 * half + mi
                        nc.tensor.matmul(
                            ps[:, mi, :],
                            w34[:, :, 128 * m : 128 * (m + 1)],
                            h2_sb[t],
                            start=True, stop=True, perf_mode=DR,
                        )
                    pss.append(ps)
                h4_sb[t] = acts.tile([128, 4, B], fp8, name="h4", tag="h4", bufs=3)
                nc.scalar.activation(
                    out=h4_sb[t][:, 0:2, :], in_=pss[0], func=AF.Relu,
                    bias=0.0, scale=1.0,
                )
                nc.vector.tensor_scalar_max(h4_sb[t][:, 2:4, :], pss[1], 0.0)
                h2_sb[t] = None

            def stageC(t):
                ps = pmm.tile([100, 2, B], f32, name="psc", tag="mm")
                for c in range(2):
                    for mp in range(2):
                        nc.tensor.matmul(
                            ps[:, mp, :],
                            w567[:, 2 * c : 2 * c + 2, 112 * mp : 112 * mp + 100],
                            h4_sb[t][:, 2 * c : 2 * c + 2, :],
                            start=(c == 0), stop=(c == 1), perf_mode=DR,
                        )
                f_sb[t] = fgp.tile([100, 2, B], fp8, name="ft", tag="f")
                nc.scalar.activation(
                    out=f_sb[t], in_=ps, func=AF.Identity, bias=0.0, scale=sC
                )
                g_sb[t] = fgp.tile([100, 2, B], fp8, name="gt", tag="g")
                nc.gpsimd.tensor_mul(g_sb[t], f_sb[t], f_sb[t])
                h4_sb[t] = None

            def stageD(t):
                ps = pdp.tile([72, B], f32, name="psd", tag="sd")
                nc.tensor.matmul(
                    ps, cm2[:, :, 0:72], f_sb[t], start=True, stop=False, perf_mode=DR
                )
                nc.tensor.matmul(
                    ps, onesd[:, :, 0:72], g_sb[t], start=False, stop=True, perf_mode=DR
                )
                sd_sb[t] = acts.tile([72, B], f32, name="sd", tag="sdp")
                nc.vector.tensor_scalar(
                    out=sd_sb[t], in0=ps, scalar1=sD, scalar2=csq[:, 0:1],
                    op0=ALU.mult, op1=ALU.add,
                )
                f_sb[t] = None
                g_sb[t] = None

            def stageT(t):
                pq = ptp.tile([P, C, 72], f32, name="pq", tag="pq")
                for s in range(C):
                    nc.tensor.transpose(
                        pq[:, s, :], sd_sb[t][:, P * s : P * (s + 1)], identf[:72, :72]
                    )
                sd_sb[t] = None
                nom_sb[t] = acts.tile([P, C, 72], f32, name="nom", tag="nom")
                nc.vector.reciprocal_approx_fast(out=nom_sb[t], in_=pq)

            def tail(t):
                nom = nom_sb[t]
                rs = acts.tile([P, C], f32, name="rs", tag="rs")
                nc.vector.reduce_sum(rs, nom, axis=AX.X)
                rr = acts.tile([P, C], f32, name="rr", tag="rr")
                nc.vector.reciprocal(rr, rs)
                rr_b = bass.AP(
                    tensor=rr.tensor,
                    offset=rr.offset,
                    ap=[rr.ap[0], rr.ap[1], [0, 72]],
                )
                qt = acts.tile([P, C, 72], f32, name="qt", tag="qt")
                nc.gpsimd.tensor_tensor(out=qt, in0=nom, in1=rr_b, op=ALU.mult)
                nc.sync.dma_start(out=q_r[t], in_=qt)
                nom_sb[t] = None

            load(0)
            load(1)
            stageA(0)
            load(2)
            stageA(1)
            for t in range(n_tiles + 4):
                if t + 3 < n_tiles:
                    load(t + 3)
                if 0 <= t - 4:
                    stageT(t - 4)
                if 0 <= t - 2 < n_tiles:
                    stageC(t - 2)
                if 0 <= t - 4:
                    tail(t - 4)
                if t < n_tiles:
                    stageB(t)
                if 0 <= t - 3 < n_tiles:
                    stageD(t - 3)
                if t + 2 < n_tiles:
                    stageA(t + 2)

    nc.compile()
    return nc


def _pow2(v):
    return float(2.0 ** np.round(np.log2(v)))


def prepare(inputs_np):
    """Host-side marshalling: merge affine chains in f64, calibrate fp8
    scales on a subsample, quantize, build per-core input maps."""
    import ml_dtypes

    bf = ml_dtypes.bfloat16
    f8 = ml_dtypes.float8_e4m3

    x = np.asarray(inputs_np["inputs"], dtype=np.float64)
    ws = [np.asarray(inputs_np[f"w{i}"], dtype=np.float64) for i in range(1, 8)]
    bs = [np.asarray(inputs_np[f"b{i}"], dtype=np.float64) for i in range(1, 8)]
    center = np.asarray(inputs_np["center"], dtype=np.float64)

    W12 = ws[0] @ ws[1]
    b12 = bs[0] @ ws[1] + bs[1]
    W34 = ws[2] @ ws[3]
    b34 = bs[2] @ ws[3] + bs[3]
    W567 = ws[4] @ ws[5] @ ws[6]
    b567 = (bs[4] @ ws[5] + bs[5]) @ ws[6] + bs[6]
    cp = center - b567[:, None]  # shifted centers c' = c - b567, [200, 72]

    n = x.shape[0]
    sub = x[:: max(1, n // 4096)][:4096]
    h2 = np.maximum(sub @ W12 + b12, 0.0)
    h4 = np.maximum(h2 @ W34 + b34, 0.0)
    e = h4 @ W567

    def rms(a):
        return float(np.sqrt(np.mean(a.astype(np.float64) ** 2)) + 1e-30)

    cA = _pow2(1.0 / rms(h2))
    kB = _pow2(0.25 / rms(W34))
    # keep the (kB*cA)-scaled h4 inside fp8 range
    while kB * cA * rms(h4) > 8.0:
        kB /= 2.0
    kC = _pow2(0.25 / rms(W567))
    cF = min(_pow2(1.0 / rms(e)), 256.0)
    sC = cF / (kC * kB * cA)
    sD = 1.0 / cF  # kD == cF

    def q8(a):
        return np.clip(a, -224.0, 224.0).astype(f8)

    # Drop the single least-impactful h2 feature to free one slot in the
    # 256-wide h2 vector; the freed slot carries a constant so b34 rides
    # through the B matmul and its epilogues need no bias columns.
    impact = np.mean(h2**2, axis=0) * np.sum(W34**2, axis=1)
    jstar = int(np.argmin(impact))
    keep = [j for j in range(256) if j != jstar]
    W12k = W12[:, keep]
    b12k = b12[keep]
    W34k = W34[keep, :]

    consts = {}
    w12t = np.zeros((73, 256), dtype=np.float64)
    w12t[:72, 0:255] = W12k
    w12t[72, 0:255] = b12k
    w12t[72, 255] = 1.0  # ones-slot: psA[255] = 1 -> h2'[255] = cA exactly
    consts["w12"] = w12t.astype(bf)
    w34f = np.zeros((256, 512), dtype=np.float64)
    w34f[0:255, :] = kB * W34k
    w34f[255, :] = kB * b34  # h2'[255] = cA  =>  contributes kB*cA*b34
    w34t = np.zeros((128, 2, 512), dtype=np.float64)
    for i in range(2):
        w34t[:, i, :] = w34f[128 * i : 128 * (i + 1), :]
    consts["w34"] = q8(w34t.reshape(128, 1024))
    w567t = np.zeros((128, 4, 224), dtype=np.float64)
    for c in range(2):
        for i in range(2):
            blk = kC * W567[256 * c + 128 * i : 256 * c + 128 * (i + 1), :]
            w567t[:, 2 * c + i, 0:100] = blk[:, 0:100]
            w567t[:, 2 * c + i, 112:212] = blk[:, 100:200]
    consts["w567"] = q8(w567t.reshape(128, 896))
    cm2t = np.zeros((100, 2, 80), dtype=np.float64)
    onest = np.zeros((100, 2, 80), dtype=np.float64)
    for i in range(2):
        cm2t[:, i, 0:72] = -2.0 * cp[100 * i : 100 * (i + 1), :]
        onest[:, i, 0:72] = 1.0 / cF
    consts["cm2"] = q8(cm2t.reshape(100, 160))
    consts["onesd"] = q8(onest.reshape(100, 160))
    consts["csq"] = (1.0 + (cp**2).sum(axis=0)).reshape(72, 1).astype(np.float32)

    n_loc = n // N_CORES
    key = (n_loc, cA, sC, sD)
    if key not in _CACHE:
        _CACHE[key] = _build(n_loc, cA, sC, sD)
    nc = _CACHE[key]

    in_maps = []
    for c in range(N_CORES):
        xt = np.empty((73, n_loc), dtype=bf)
        xt[:72] = x[c * n_loc : (c + 1) * n_loc].T
        xt[72] = 1.0
        m = {"xt": np.ascontiguousarray(xt)}
        m.update(consts)
        in_maps.append(m)
    return nc, in_maps


def kernel(
    inputs, w1, b1, w2, b2, w3, b3, w4, b4, w5, b5, w6, b6, w7, b7, center
):
    from concourse.bass_utils import run_bass_kernel_spmd

    inputs_np = {
        "inputs": inputs, "center": center,
        "w1": w1, "b1": b1, "w2": w2, "b2": b2, "w3": w3, "b3": b3,
        "w4": w4, "b4": b4, "w5": w5, "b5": b5, "w6": w6, "b6": b6,
        "w7": w7, "b7": b7,
    }
    nc, in_maps = prepare(inputs_np)
    res = run_bass_kernel_spmd(nc, in_maps, core_ids=list(range(N_CORES)))
    return np.concatenate([res.results[c]["q"] for c in range(N_CORES)], axis=0)


# revision 18
# speedup vs baseline: 1.9642x; 1.0239x over previous
"""Trainium2 Bass kernel for nn_DeepCluster (vq_codebook).

Math (per row x in R^72):
  7-layer MLP, ReLU only after layers 2 and 4  ->  f in R^200
  sq[j] = |f - center[:, j]|^2 ;  q = (1/(1+sq)) / sum_j (1/(1+sq))

Because ReLU appears only after layers 2 and 4, the layer chains 1-2,
3-4 and 5-6-7 are affine compositions.  The host pre-multiplies them in
float64 into three matrices W12 [72,256], W34 [256,512], W567 [512,200]
(3.25x fewer matmul FLOPs than the literal 7-layer chain).  The device
then computes, per 512-row tile (feature-major layout [feat, batch]):

  A: h2 = relu(x @ W12 + b12)        2 bf16 matmuls (bias via ones-row)
  B: h4 = relu(h2 @ W34 + b34)       4 fp8 DoubleRow matmuls (K=256)
  C: e  = h4 @ W567                  4 fp8 DoubleRow matmuls (K=512)
  D: sq = |e|^2 - 2 e.(c-b567) ...   2 fp8 DoubleRow matmuls (K=200)
  tail: 1/(1+sq), transpose back, row-normalize, store

The C-layer bias b567 is folded into shifted centers c' = c - b567 and
|c'|^2, so B is the only stage needing per-feature bias epilogues.
Activations are scaled by per-stage powers of two (calibrated on a host
subsample) to sit in fp8e4 range; all scale folds are exact.
Work is software-pipelined 3 tiles deep and spread over ACT/DVE/GPSIMD
so the PE (12 matmuls + 4 transposes per tile) stays the critical path.
"""

import numpy as np

N_CORES = 8
B = 512  # rows per pipeline tile
P = 128

_CACHE = {}


def _build(n_rows, cA, sC, sD):
    import concourse.bass as bass
    import concourse.mybir as mybir
    from concourse import bacc
    from concourse.tile import TileContext
    from concourse.masks import make_identity

    f32 = mybir.dt.float32
    bf16 = mybir.dt.bfloat16
    fp8 = mybir.dt.float8e4
    AF = mybir.ActivationFunctionType
    AX = mybir.AxisListType
    ALU = mybir.AluOpType
    DR = mybir.MatmulPerfMode.DoubleRow

    nc = bacc.Bacc(None, target_bir_lowering=False, debug=False)
    xt_d = nc.dram_tensor("xt", [73, n_rows], bf16, kind="ExternalInput")
    q_d = nc.dram_tensor("q", [n_rows, 72], f32, kind="ExternalOutput")
    w12_d = nc.dram_tensor("w12", [73, 256], bf16, kind="ExternalInput")
    w34_d = nc.dram_tensor("w34", [128, 1024], fp8, kind="ExternalInput")
    w567_d = nc.dram_tensor("w567", [128, 896], fp8, kind="ExternalInput")
    cm2_d = nc.dram_tensor("cm2", [100, 160], fp8, kind="ExternalInput")
    onesd_d = nc.dram_tensor("onesd", [100, 160], fp8, kind="ExternalInput")
    csq_d = nc.dram_tensor("csq", [72, 1], f32, kind="ExternalInput")

    n_tiles = n_rows // B
    assert n_rows % B == 0
    C = B // P

    with TileContext(nc) as tc:
        with (
            tc.tile_pool(name="consts", bufs=1) as consts,
            tc.tile_pool(name="xt", bufs=4) as xtp,
            tc.tile_pool(name="acts", bufs=3) as acts,
            tc.tile_pool(name="h2p", bufs=3) as h2p,
            tc.tile_pool(name="fg", bufs=5) as fgp,
            tc.tile_pool(name="pmm", bufs=3, space="PSUM") as pmm,
            tc.tile_pool(name="pd", bufs=1, space="PSUM") as pdp,
            tc.tile_pool(name="pt", bufs=1, space="PSUM") as ptp,
        ):
            identf = consts.tile([128, 128], f32, tag="identf")
            make_identity(nc, identf)
            w12 = consts.tile([73, 256], bf16, tag="w12")
            nc.sync.dma_start(out=w12, in_=w12_d[:])
            w34 = consts.tile([128, 2, 512], fp8, tag="w34")
            nc.sync.dma_start(out=w34, in_=w34_d[:].rearrange("p (i m) -> p i m", i=2))
            w567 = consts.tile([128, 4, 224], fp8, tag="w567")
            nc.sync.dma_start(
                out=w567, in_=w567_d[:].rearrange("p (i m) -> p i m", i=4)
            )
            cm2 = consts.tile([100, 2, 80], fp8, tag="cm2")
            nc.sync.dma_start(out=cm2, in_=cm2_d[:].rearrange("p (i m) -> p i m", i=2))
            onesd = consts.tile([100, 2, 80], fp8, tag="onesd")
            nc.sync.dma_start(
                out=onesd, in_=onesd_d[:].rearrange("p (i m) -> p i m", i=2)
            )
            csq = consts.tile([72, 1], f32, tag="csq")
            nc.sync.dma_start(out=csq, in_=csq_d[:])

            q_r = q_d[:].rearrange("(t s p) j -> t p s j", p=P, s=C)

            xt_sb = [None] * n_tiles
            h2_sb = [None] * n_tiles
            h4_sb = [None] * n_tiles
            f_sb = [None] * n_tiles
            g_sb = [None] * n_tiles
            sd_sb = [None] * n_tiles
            nom_sb = [None] * n_tiles

            def load(t):
                xt_sb[t] = xtp.tile([73, B], bf16, name="xt", tag="x")
                nc.sync.dma_start(out=xt_sb[t], in_=xt_d[:, B * t : B * (t + 1)])

            def stageA(t):
                ps = pmm.tile([128, 2, B], f32, name="psmm", tag="mm")
                for m in range(2):
                    nc.tensor.matmul(
                        ps[:, m, :], w12[:, 128 * m : 128 * (m + 1)], xt_sb[t],
                        start=True, stop=True,
                    )
                h2_sb[t] = h2p.tile([128, 2, B], fp8, name="h2", tag="h2")
                nc.scalar.activation(
                    out=h2_sb[t], in_=ps, func=AF.Relu, bias=0.0, scale=cA
                )
                xt_sb[t] = None

            def stageB(t):
                pss = []
                for half in range(2):
                    ps = pmm.tile([128, 2, B], f32, name="psmm", tag="mm")
                    for mi in range(2):
                        m = 2 * half + mi
                        nc.tensor.matmul(
                            ps[:, mi, :],
                            w34[:, :, 128 * m : 128 * (m + 1)],
                            h2_sb[t],
                            start=True, stop=True, perf_mode=DR,
                        )
                    pss.append(ps)
                h4_sb[t] = acts.tile([128, 4, B], fp8, name="h4", tag="h4", bufs=3)
                nc.scalar.activation(
                    out=h4_sb[t][:, 0:2, :], in_=pss[0], func=AF.Relu,
                    bias=0.0, scale=1.0,
                )
                nc.vector.tensor_scalar_max(h4_sb[t][:, 2:4, :], pss[1], 0.0)
                h2_sb[t] = None

            def stageC(t):
                ps = pmm.tile([100, 2, B], f32, name="psc", tag="mm")
                for c in range(2):
                    for mp in range(2):
                        nc.tensor.matmul(
                            ps[:, mp, :],
                            w567[:, 2 * c : 2 * c + 2, 112 * mp : 112 * mp + 100],
                            h4_sb[t][:, 2 * c : 2 * c + 2, :],
                            start=(c == 0), stop=(c == 1), perf_mode=DR,
                        )
                f_sb[t] = fgp.tile([100, 2, B], fp8, name="ft", tag="f")
                nc.scalar.activation(
                    out=f_sb[t], in_=ps, func=AF.Identity, bias=0.0, scale=sC
                )
                g_sb[t] = fgp.tile([100, 2, B], fp8, name="gt", tag="g")
                nc.gpsimd.tensor_mul(
                    g_sb[t][:, 0, :], f_sb[t][:, 0, :], f_sb[t][:, 0, :]
                )
                nc.vector.tensor_mul(
                    g_sb[t][:, 1, :], f_sb[t][:, 1, :], f_sb[t][:, 1, :]
                )
                h4_sb[t] = None

            def stageD(t):
                ps = pdp.tile([72, B], f32, name="psd", tag="sd")
                nc.tensor.matmul(
                    ps, cm2[:, :, 0:72], f_sb[t], start=True, stop=False, perf_mode=DR
                )
                nc.tensor.matmul(
                    ps, onesd[:, :, 0:72], g_sb[t], start=False, stop=True, perf_mode=DR
                )
                sd_sb[t] = acts.tile([72, B], f32, name="sd", tag="sdp")
                nc.vector.tensor_scalar(
                    out=sd_sb[t], in0=ps, scalar1=sD, scalar2=csq[:, 0:1],
                    op0=ALU.mult, op1=ALU.add,
                )
                f_sb[t] = None
                g_sb[t] = None

            def stageT(t):
                pq = ptp.tile([P, C, 72], f32, name="pq", tag="pq")
                for s in range(C):
                    nc.tensor.transpose(
                        pq[:, s, :], sd_sb[t][:, P * s : P * (s + 1)], identf[:72, :72]
                    )
                sd_sb[t] = None
                nom_sb[t] = acts.tile([P, C, 72], f32, name="nom", tag="nom")
                nc.vector.reciprocal_approx_fast(out=nom_sb[t], in_=pq)

            def tail(t):
                nom = nom_sb[t]
                rs = acts.tile([P, C], f32, name="rs", tag="rs")
                nc.vector.reduce_sum(rs, nom, axis=AX.X)
                rr = acts.tile([P, C], f32, name="rr", tag="rr")
                nc.vector.reciprocal(rr, rs)
                rr_b = bass.AP(
                    tensor=rr.tensor,
                    offset=rr.offset,
                    ap=[rr.ap[0], rr.ap[1], [0, 72]],
                )
                qt = acts.tile([P, C, 72], f32, name="qt", tag="qt")
                nc.gpsimd.tensor_tensor(out=qt, in0=nom, in1=rr_b, op=ALU.mult)
                nc.sync.dma_start(out=q_r[t], in_=qt)
                nom_sb[t] = None

            load(0)
            load(1)
            stageA(0)
            load(2)
            stageA(1)
            for t in range(n_tiles + 4):
                if t + 3 < n_tiles:
                    load(t + 3)
                if 0 <= t - 4:
                    stageT(t - 4)
                if 0 <= t - 2 < n_tiles:
                    stageC(t - 2)
                if 0 <= t - 4:
                    tail(t - 4)
                if t < n_tiles:
                    stageB(t)
                if 0 <= t - 3 < n_tiles:
                    stageD(t - 3)
                if t + 2 < n_tiles:
                    stageA(t + 2)

    nc.compile()
    return nc


def _pow2(v):
    return float(2.0 ** np.round(np.log2(v)))


def prepare(inputs_np):
    """Host-side marshalling: merge affine chains in f64, calibrate fp8
    scales on a subsample, quantize, build per-core input maps."""
    import ml_dtypes

    bf = ml_dtypes.bfloat16
    f8 = ml_dtypes.float8_e4m3

    x = np.asarray(inputs_np["inputs"], dtype=np.float64)
    ws = [np.asarray(inputs_np[f"w{i}"], dtype=np.float64) for i in range(1, 8)]
    bs = [np.asarray(inputs_np[f"b{i}"], dtype=np.float64) for i in range(1, 8)]
    center = np.asarray(inputs_np["center"], dtype=np.float64)

    W12 = ws[0] @ ws[1]
    b12 = bs[0] @ ws[1] + bs[1]
    W34 = ws[2] @ ws[3]
    b34 = bs[2] @ ws[3] + bs[3]
    W567 = ws[4] @ ws[5] @ ws[6]
    b567 = (bs[4] @ ws[5] + bs[5]) @ ws[6] + bs[6]
    cp = center - b567[:, None]  # shifted centers c' = c - b567, [200, 72]

    n = x.shape[0]
    sub = x[:: max(1, n // 4096)][:4096]
    h2 = np.maximum(sub @ W12 + b12, 0.0)
    h4 = np.maximum(h2 @ W34 + b34, 0.0)
    e = h4 @ W567

    def rms(a):
        return float(np.sqrt(np.mean(a.astype(np.float64) ** 2)) + 1e-30)

    cA = _pow2(1.0 / rms(h2))
    kB = _pow2(0.25 / rms(W34))
    # keep the (kB*cA)-scaled h4 inside fp8 range
    while kB * cA * rms(h4) > 8.0:
        kB /= 2.0
    kC = _pow2(0.25 / rms(W567))
    cF = min(_pow2(1.0 / rms(e)), 256.0)
    sC = cF / (kC * kB * cA)
    sD = 1.0 / cF  # kD == cF

    def q8(a):
        return np.clip(a, -224.0, 224.0).astype(f8)

    # Drop the single least-impactful h2 feature to free one slot in the
    # 256-wide h2 vector; the freed slot carries a constant so b34 rides
    # through the B matmul and its epilogues need no bias columns.
    impact = np.mean(h2**2, axis=0) * np.sum(W34**2, axis=1)
    jstar = int(np.argmin(impact))
    keep = [j for j in range(256) if j != jstar]
    W12k = W12[:, keep]
    b12k = b12[keep]
    W34k = W34[keep, :]

    consts = {}
    w12t = np.zeros((73, 256), dtype=np.float64)
    w12t[:72, 0:255] = W12k
    w12t[72, 0:255] = b12k
    w12t[72, 255] = 1.0  # ones-slot: psA[255] = 1 -> h2'[255] = cA exactly
    consts["w12"] = w12t.astype(bf)
    w34f = np.zeros((256, 512), dtype=np.float64)
    w34f[0:255, :] = kB * W34k
    w34f[255, :] = kB * b34  # h2'[255] = cA  =>  contributes kB*cA*b34
    w34t = np.zeros((128, 2, 512), dtype=np.float64)
    for i in range(2):
        w34t[:, i, :] = w34f[128 * i : 128 * (i + 1), :]
    consts["w34"] = q8(w34t.reshape(128, 1024))
    w567t = np.zeros((128, 4, 224), dtype=np.float64)
    for c in range(2):
        for i in range(2):
            blk = kC * W567[256 * c + 128 * i : 256 * c + 128 * (i + 1), :]
            w567t[:, 2 * c + i, 0:100] = blk[:, 0:100]
            w567t[:, 2 * c + i, 112:212] = blk[:, 100:200]
    consts["w567"] = q8(w567t.reshape(128, 896))
    cm2t = np.zeros((100, 2, 80), dtype=np.float64)
    onest = np.zeros((100, 2, 80), dtype=np.float64)
    for i in range(2):
        cm2t[:, i, 0:72] = -2.0 * cp[100 * i : 100 * (i + 1), :]
        onest[:, i, 0:72] = 1.0 / cF
    consts["cm2"] = q8(cm2t.reshape(100, 160))
    consts["onesd"] = q8(onest.reshape(100, 160))
    consts["csq"] = (1.0 + (cp**2).sum(axis=0)).reshape(72, 1).astype(np.float32)

    n_loc = n // N_CORES
    key = (n_loc, cA, sC, sD)
    if key not in _CACHE:
        _CACHE[key] = _build(n_loc, cA, sC, sD)
    nc = _CACHE[key]

    in_maps = []
    for c in range(N_CORES):
        xt = np.empty((73, n_loc), dtype=bf)
        xt[:72] = x[c * n_loc : (c + 1) * n_loc].T
        xt[72] = 1.0
        m = {"xt": np.ascontiguousarray(xt)}
        m.update(consts)
        in_maps.append(m)
    return nc, in_maps


def kernel(
    inputs, w1, b1, w2, b2, w3, b3, w4, b4, w5, b5, w6, b6, w7, b7, center
):
    from concourse.bass_utils import run_bass_kernel_spmd

    inputs_np = {
        "inputs": inputs, "center": center,
        "w1": w1, "b1": b1, "w2": w2, "b2": b2, "w3": w3, "b3": b3,
        "w4": w4, "b4": b4, "w5": w5, "b5": b5, "w6": w6, "b6": b6,
        "w7": w7, "b7": b7,
    }
    nc, in_maps = prepare(inputs_np)
    res = run_bass_kernel_spmd(nc, in_maps, core_ids=list(range(N_CORES)))
    return np.concatenate([res.results[c]["q"] for c in range(N_CORES)], axis=0)
